# revision 11
# baseline (speedup 1.0000x reference)
"""ContrastiveLoss kernel for 8x TRN2 NeuronCores (Bass/Tile).

Math: z2 = cat(positives, negatives) [n, m, d]; a_n, z2_n = L2-normalized;
logits = (a_n @ z2_n^T) / T  [n, n*m]; labels = eye(n) broadcast over m.
BCEWithLogits mean  =  ( sum softplus(logits) - sum_diag logits ) / (n*n*m)

Sharding: z2 rows (j axis) split across 8 cores -> each core holds
z slab [256*16, 128] and the full anchor, computes its [n, 4096] slab of
logits fused with the softplus+reduce epilogue (never materialized to HBM),
plus the diagonal-logit partial sum. Host adds 8 scalar partials.

The anchor passed to core c is np.roll(anchor, -c*256, axis=0) so that the
diagonal logits for core c's j range land at core-independent psum columns
(SPMD: all cores run the identical program).
"""

import json as _json
import re as _re

import numpy as np

import concourse.bass as bass
import concourse.bass2jax as _bass2jax
import concourse.tile as tile
from concourse import mybir
from concourse.bass_utils import run_bass_kernel_spmd
from concourse.vector_clock import ScopedClock, VectorClock


def _split_multi_waits_bir(bir_bytes):
    """This walrus build allows at most one sync wait per instruction
    (setupSyncWait: "Too many sync wait commands"). Hoist extra waits onto
    standalone EventSemaphore instructions (the wait_ge encoding) emitted
    just before the instruction on the same engine queue — semantically
    identical under in-order engine execution."""
    d = _json.loads(bir_bytes)
    for fn in d.get("functions", []):
        for bb in fn.get("blocks", []):
            insts = bb.get("instructions", [])
            out = []
            for inst in insts:
                si = inst.get("sync_info")
                if si:
                    waits = si.get("on_wait") or []
                    if len(waits) > 1:
                        for k, w in enumerate(waits[:-1]):
                            out.append({
                                "debug": inst.get("debug", 0),
                                "engine": inst["engine"],
                                "ins": [],
                                "outs": [],
                                "name": f"{inst['name']}-xw{k}",
                                "opcode": "EventSemaphore",
                                "sync_info": {"on_update": [], "on_wait": [w]},
                            })
                        si["on_wait"] = [waits[-1]]
                out.append(inst)
            bb["instructions"] = out
    return _json.dumps(d).encode()


_orig_compile_bir_kernel = _bass2jax.compile_bir_kernel


def _compile_bir_kernel_splitwaits(ant_bir_str, *args, **kwargs):
    if isinstance(ant_bir_str, str):
        ant_bir_str = ant_bir_str.encode()
    return _orig_compile_bir_kernel(
        _split_multi_waits_bir(ant_bir_str), *args, **kwargs
    )


if _bass2jax.compile_bir_kernel is not _compile_bir_kernel_splitwaits:
    _bass2jax.compile_bir_kernel = _compile_bir_kernel_splitwaits


class _TC(tile.TileContext):
    """TileContext whose tail drain uses one single-wait Drain per pending
    proc: this walrus build rejects a Drain carrying more than one sync wait
    ("Too many sync wait commands", CoreV3GenImpl setupSyncWait)."""

    def _drain_and_barrier(self, tick_clock, wait_clock):
        gc = tick_clock.global_clock
        ticks = [int(s) for s in _re.findall(r"\d+", repr(gc))]
        for proc, tick in enumerate(ticks):
            if tick > 0:
                vc = VectorClock()
                vc.require_at_least(proc, tick)
                d = self.nc.sync.drain()
                wait_clock.add_sem_waits(d.ins, ScopedClock({None: vc}))
        self.nc.all_engine_barrier()
        popped = self.nc._tile_sem_poison_stack.pop()
        assert popped is self._sem_poison
        self.nc.clear_and_free_semaphores(list(self.sems.allocated().values()))
        self.nc.all_engine_barrier()

N = 2048          # anchor rows
D = 128           # feature dim
M = 16            # candidates per row (1 pos + 15 neg)
NCORES = 8
JS = N // NCORES  # 256 j rows per core
ZR = JS * M       # 4096 z rows per core
NT_A = N // 128   # 16 anchor tiles
NT_Z = ZR // 128  # 32 z tiles (each = 8 j * 16 m rows)
TEMPERATURE = 0.3
INV_T = 1.0 / TEMPERATURE

FP32 = mybir.dt.float32
BF16 = mybir.dt.bfloat16
U32 = mybir.dt.uint32
FP16 = mybir.dt.float16

_NC_CACHE = None
LAST_RESULTS = None


# bitcast(bits(x)>>1) = sqrt(x) * 2^62 / RSQRT_C2 within +-3%; then Newton.
RSQRT_C2 = 7.8955675e-20


def _emit_rsqrt(nc, ssq, rs, tmp, ncols, newton_iters=3):
    """rs[:, :ncols] = 1/sqrt(ssq[:, :ncols]); DVE-only (shift seed + Newton)."""
    s_ssq = ssq[:, 0:ncols]
    s_rs = rs[:, 0:ncols]
    s_tmp = tmp[:, 0:ncols]
    # tmp = bitcast(bits(ssq) >> 1) ~ sqrt(ssq)*2^62/c2
    nc.vector.tensor_scalar(
        out=s_tmp.bitcast(U32), in0=s_ssq.bitcast(U32), scalar1=1, scalar2=None,
        op0=mybir.AluOpType.logical_shift_right,
    )
    # rs = c2 * reciprocal(tmp) ~ rsqrt(ssq) to ~3%
    nc.vector.reciprocal(out=s_rs, in_=s_tmp)
    nc.vector.tensor_scalar(
        out=s_rs, in0=s_rs, scalar1=RSQRT_C2, scalar2=None,
        op0=mybir.AluOpType.mult,
    )
    for _ in range(newton_iters):
        # rs = rs * (1.5 - 0.5 * ssq * rs^2)
        nc.vector.tensor_mul(out=s_tmp, in0=s_rs, in1=s_rs)
        nc.vector.tensor_mul(out=s_tmp, in0=s_tmp, in1=s_ssq)
        nc.vector.tensor_scalar(
            out=s_tmp, in0=s_tmp, scalar1=-0.5, scalar2=1.5,
            op0=mybir.AluOpType.mult, op1=mybir.AluOpType.add,
        )
        nc.vector.tensor_mul(out=s_rs, in0=s_rs, in1=s_tmp)


def _build_nc():
    nc = bass.Bass()
    anchor = nc.declare_dram_parameter("anchor", [N, D], FP32, isOutput=False)
    z2 = nc.declare_dram_parameter("z2", [ZR, D], FP32, isOutput=False)
    mask = nc.declare_dram_parameter("mask", [128, 8], FP32, isOutput=False)
    out = nc.declare_dram_parameter("out", [128, 2], FP32, isOutput=True)

    ZG = 8              # z rsqrt batching group size (tiles)
    n_zg = NT_Z // ZG   # 4 groups

    with _TC(nc) as tc:
        with (
            tc.tile_pool(name="araw", bufs=NT_A) as araw_pool,
            tc.tile_pool(name="zraw", bufs=ZG + 4) as zraw_pool,
            tc.tile_pool(name="nbf", bufs=8) as nbf_pool,
            tc.tile_pool(name="big", bufs=1) as big,
            tc.tile_pool(name="zt", bufs=NT_Z) as zt_pool,
            tc.tile_pool(name="sg", bufs=3) as sg_pool,
            tc.tile_pool(name="t1", bufs=3) as t1_pool,
            tc.tile_pool(name="t2", bufs=3) as t2_pool,
            tc.tile_pool(name="psum", bufs=2, space="PSUM") as psum_pool,
        ):
            aT = big.tile([128, N], BF16, tag="aT")
            ssq_a = big.tile([128, NT_A], FP32, tag="ssq_a")
            rs_a = big.tile([128, NT_A], FP32, tag="rs_a")
            tmp_a = big.tile([128, NT_A], FP32, tag="tmp_a")
            ssq_z = big.tile([128, NT_Z], FP32, tag="ssq_z")
            rs_z = big.tile([128, NT_Z], FP32, tag="rs_z")
            tmp_z = big.tile([128, NT_Z], FP32, tag="tmp_z")
            diag_cols = big.tile([128, NT_Z], FP32, tag="diag_cols")
            mask_sb = big.tile([128, 8], FP32, tag="mask_sb")
            sq_scr = big.tile([128, 128], FP32, tag="sq_scr")
            dg_scr = big.tile([128, 8], FP32, tag="dg_scr")
            out_sb = big.tile([128, 2], FP32, tag="out_sb")
            # grouped sigmoid-products (K=8): ln(prod) summed at the end
            plog = big.tile([128, NT_Z * 256], FP32, tag="plog")

            nc.sync.dma_start(out=mask_sb, in_=mask[:, :])

            # ---- anchor prep: load, sumsq, rsqrt, normalize->bf16, transpose
            a_raw = []
            for t in range(NT_A):
                r = araw_pool.tile([128, 128], FP32, tag="araw")
                nc.sync.dma_start(out=r, in_=anchor[t * 128:(t + 1) * 128, :])
                nc.vector.tensor_mul(out=sq_scr, in0=r, in1=r)
                nc.vector.tensor_reduce(
                    out=ssq_a[:, t:t + 1], in_=sq_scr,
                    axis=mybir.AxisListType.X, op=mybir.AluOpType.add,
                )
                a_raw.append(r)
            _emit_rsqrt(nc, ssq_a, rs_a, tmp_a, NT_A)
            for t in range(NT_A):
                nb = nbf_pool.tile([128, 128], BF16, tag="nbf")
                nc.vector.tensor_scalar_mul(
                    out=nb, in0=a_raw[t], scalar1=rs_a[:, t:t + 1]
                )
                nc.sync.dma_start(
                    out=aT[:, t * 128:(t + 1) * 128], in_=nb, transpose=True
                )

            # ---- z prep (grouped) fused with GEMM + softplus epilogue
            for g in range(n_zg):
                u0 = g * ZG
                z_raw = []
                for u in range(u0, u0 + ZG):
                    r = zraw_pool.tile([128, 128], FP32, tag="zraw")
                    nc.sync.dma_start(out=r, in_=z2[u * 128:(u + 1) * 128, :])
                    nc.vector.tensor_mul(out=sq_scr, in0=r, in1=r)
                    nc.vector.tensor_reduce(
                        out=ssq_z[:, u:u + 1], in_=sq_scr,
                        axis=mybir.AxisListType.X, op=mybir.AluOpType.add,
                    )
                    z_raw.append(r)
                _emit_rsqrt(
                    nc, ssq_z[:, u0:u0 + ZG], rs_z[:, u0:u0 + ZG],
                    tmp_z[:, u0:u0 + ZG], ZG,
                )
                for u in range(u0, u0 + ZG):
                    nb = nbf_pool.tile([128, 128], BF16, tag="nbf")
                    nc.vector.tensor_scalar_mul(
                        out=nb, in0=z_raw[u - u0], scalar1=rs_z[:, u:u + 1]
                    )
                    zt = zt_pool.tile([128, 128], BF16, tag="zt")
                    nc.sync.dma_start(out=zt, in_=nb, transpose=True)

                    # GEMM for jm chunk u: psum[jm, i] over all 2048 anchors
                    ps = psum_pool.tile([128, N], FP32, tag="ps")
                    for v in range(N // 512):
                        nc.tensor.matmul(
                            out=ps[:, v * 512:(v + 1) * 512],
                            lhsT=zt[:, :],
                            rhs=aT[:, v * 512:(v + 1) * 512],
                            start=True, stop=True,
                        )
                    # diagonal logits (pre-scale): partition p <-> jm = u*128+p,
                    # j = u*8 + p//16, anchor col i = j (anchor rolled on host).
                    nc.vector.tensor_mul(
                        out=dg_scr, in0=ps[:, u * 8:u * 8 + 8], in1=mask_sb
                    )
                    nc.vector.tensor_reduce(
                        out=diag_cols[:, u:u + 1], in_=dg_scr,
                        axis=mybir.AxisListType.X, op=mybir.AluOpType.add,
                    )
                    # softplus(x) = -ln(sigmoid(-x)); this walrus build has no
                    # softplus ACT table, so: sigmoid pass -> K=8 pairwise
                    # products -> one batched ln+accum at the end.
                    sg = sg_pool.tile([128, N], FP16, tag="sg")
                    nc.scalar.activation(
                        out=sg, in_=ps[:, :],
                        func=mybir.ActivationFunctionType.Sigmoid,
                        scale=-INV_T,
                    )
                    t1 = t1_pool.tile([128, N // 2], FP16, tag="t1")
                    nc.gpsimd.tensor_mul(
                        out=t1, in0=sg[:, 0:N // 2], in1=sg[:, N // 2:N]
                    )
                    t2 = t2_pool.tile([128, N // 4], FP16, tag="t2")
                    nc.vector.tensor_mul(
                        out=t2, in0=t1[:, 0:N // 4], in1=t1[:, N // 4:N // 2]
                    )
                    nc.vector.tensor_mul(
                        out=plog[:, u * 256:(u + 1) * 256],
                        in0=t2[:, 0:N // 8], in1=t2[:, N // 8:N // 4],
                    )

            # ln of all grouped products, accumulated per partition
            nc.scalar.activation(
                out=plog, in_=plog,
                func=mybir.ActivationFunctionType.Ln,
                accum_out=out_sb[:, 0:1],
            )
            nc.vector.tensor_reduce(
                out=out_sb[:, 1:2], in_=diag_cols,
                axis=mybir.AxisListType.X, op=mybir.AluOpType.add,
            )
            nc.sync.dma_start(out=out[:, :], in_=out_sb)

    return nc


def _get_nc():
    global _NC_CACHE
    if _NC_CACHE is None:
        _NC_CACHE = _build_nc()
    return _NC_CACHE


def kernel(anchor, positives, negatives):
    global LAST_RESULTS
    anchor = np.ascontiguousarray(np.asarray(anchor, dtype=np.float32))
    positives = np.asarray(positives, dtype=np.float32)
    negatives = np.asarray(negatives, dtype=np.float32)

    z2 = np.concatenate([positives, negatives], axis=1)  # [N, M, D]

    mask = np.zeros((128, 8), dtype=np.float32)
    for p in range(128):
        mask[p, p // 16] = 1.0

    in_maps = []
    for c in range(NCORES):
        slab = np.ascontiguousarray(
            z2[c * JS:(c + 1) * JS].reshape(ZR, D)
        )
        a_c = np.ascontiguousarray(np.roll(anchor, -c * JS, axis=0))
        in_maps.append({"anchor": a_c, "z2": slab, "mask": mask})

    nc = _get_nc()
    res = run_bass_kernel_spmd(nc, in_maps, list(range(NCORES)))
    LAST_RESULTS = res

    lnsig_total = 0.0
    diag_total = 0.0
    for c in range(NCORES):
        o = res.results[c]["out"].astype(np.float64)
        lnsig_total += o[:, 0].sum()
        diag_total += o[:, 1].sum()

    # sum softplus(x) = -sum ln(sigmoid(-x)) = -lnsig_total
    loss = (-lnsig_total - diag_total * INV_T) / (N * N * M)
    return np.float32(loss)
